# revision 46
# baseline (speedup 1.0000x reference)
"""Trainium2 Bass kernel for an 8-batch single-head attention block.

Reference computation (per batch b of 8, S=2048 seq, D=A=768):
    Q = relu(X Wq + bq); K = relu(X Wk + bk); V = relu(X Wv + bv)
    P = softmax(Q K^T)          (no 1/sqrt(d) scale)
    X1 = LN(X + P V)
    X2 = LN(X1 + X1 Wd + bd)    (LN affines are identity in this problem)

Sharding: data-parallel — batch b -> NeuronCore b (8 cores, no collectives).

Design notes (each validated against neuron-profile traces; 478us -> 330us):
  * All matmul operands are fp16: the PE multiplies bf16/fp16/fp32r at the
    same FP22 internal precision and 1 cycle/row, but fp32r weight loads
    (4B) take ~213ns vs ~107ns for 16-bit -- with an LDWEIGHTS per matmul
    the fp32r version was LDW-throughput-bound (~265ns issue period vs the
    213ns stream floor). fp16 keeps fp32r-level precision (its 11-bit
    mantissa maps into FP22 exactly) at half the LDW/DMA cost. E^T stays
    bf16 (unscaled exp needs the e8 range); V bf16 to match the attn MM.
    (fp8e4 DoubleRow for the attn matmul was simulated host-side: ~4.5%
    error on attn out ~= the whole 2e-2 error budget -> rejected.)
  * Q^T and K^T are both kept resident in SBUF (no DRAM spill round-trip).
  * Matmul PSUM outputs are capped at one 2KB bank (512 f32) -- wider
    outputs compile but fail at NEFF build (verified on hw).
  * Phase B runs d-OUTER with one psum bank per output e-block, so the
    first matmuls issue as soon as the first weight/xt tiles land; weights
    are packed per d-block into wide DRAM rows (wk|wq, wv|wd -> 3KB DMA
    lines, ~2x the startup DMA rate of 1.5KB lines), and startup uses two
    DMA queues in parallel (sync: weights, scalar: xt chunk 0).
  * Biases: bq/bk ride the ACT relu bias port (per-partition, since K^T/Q^T
    are transposed); bv/bd are partition-broadcast once by gpsimd and added
    on DVE (cheaper than K=1 ones-row matmuls on the PE-bound stream).
  * LayerNorm math runs on DVE via tensor_scalar (two scalar operands); a
    centered-accumulate trick gets D*var in the same pass that applies the
    residual; ACT only does Relu / Exp / Sqrt (Relu+Exp share one table
    load; Sqrt reloads are off the critical path).
  * Phase C is software-pipelined per q-block: attn(qs+1) covers LN1(qs),
    attn covers each transpose->copy latency, and scores(c+1) is issued
    before the last transpose/dense pair of chunk c so the PE never drains
    while DVE chains complete. The rowsum half (pa1) of each attn q-block
    is computed first so the reciprocal overlaps the pa0 matmul stream.
    (Deferring all dense stages to a packed tail was tried and reverted:
    it also defers all output DMA into the last 45us, which drains slower
    than the HAM-throttle it avoids.)
"""

from contextlib import ExitStack

import numpy as np

import concourse.bass as bass
import concourse.mybir as mybir
import concourse.tile as tile
from concourse import bacc
from concourse.bass_utils import run_bass_kernel_spmd
from concourse.masks import make_identity

S, D = 2048, 768
N_CORES = 8
SB, DB = S // 128, D // 128  # 16 s-blocks, 6 d-blocks
SCH = 512   # phase-B s-chunk width
QCH = 512   # phase-C q-chunk width
NCH = S // SCH
F32 = mybir.dt.float32
F16 = mybir.dt.float16
BF16 = mybir.dt.bfloat16
ET_DT = BF16
V_DT = BF16
AF = mybir.ActivationFunctionType
ALU = mybir.AluOpType
EPS = 1e-5


def _split_matmul_waits(nc):
    """Walrus allows only one semaphore wait on self-loading (fp32/fp32r/
    transpose) Matmult instructions; move extra waits onto a preceding
    InstEventSemaphore (which may carry two waits each)."""
    for bb in nc.main_func.blocks:
        new_insts = []
        for inst in bb.instructions:
            if isinstance(inst, mybir.InstMatmult) and inst.sync_info is not None \
                    and len(inst.sync_info.on_wait) > 1:
                waits = list(inst.sync_info.on_wait)
                extra, keep = waits[:-1], waits[-1:]
                while extra:
                    chunk, extra = extra[:2], extra[2:]
                    ev = mybir.InstEventSemaphore(
                        name=nc.get_next_instruction_name(), ins=[], outs=[])
                    ev.engine = inst.engine
                    ev.sync_info = mybir.SyncInfo(on_wait=chunk, on_update=[])
                    nc.register_instruction(ev)
                    new_insts.append(ev)
                inst.sync_info = mybir.SyncInfo(
                    on_wait=keep, on_update=list(inst.sync_info.on_update))
            new_insts.append(inst)
        bb.instructions[:] = new_insts


def _build():
    nc = bacc.Bacc("TRN2", target_bir_lowering=False, debug=False,
                   enable_asserts=False, num_devices=N_CORES)

    # weights packed per d-block into wide rows (3KB lines DMA much faster
    # than 1.5KB): wkq = wk|wq, wvd = wv_aug|wd
    xt_d = nc.dram_tensor("xt", [DB, 128, S], F16, kind="ExternalInput").ap()
    xr_d = nc.dram_tensor("xr", [S, D], F16, kind="ExternalInput").ap()
    wkq_d = nc.dram_tensor("wkq", [DB, 128, 2 * D], F16, kind="ExternalInput").ap()
    wvd_d = nc.dram_tensor("wvd", [DB, 128, 2 * D + 2], F16,
                           kind="ExternalInput").ap()
    bqk_d = nc.dram_tensor("bqk", [128, 2 * DB], F32, kind="ExternalInput").ap()
    bv_d = nc.dram_tensor("bv", [1, D + 2], F16, kind="ExternalInput").ap()
    bd_d = nc.dram_tensor("bd", [1, D], F16, kind="ExternalInput").ap()
    out_d = nc.dram_tensor("out", [S, D], F32, kind="ExternalOutput").ap()

    with tile.TileContext(nc) as tc, ExitStack() as ctx:
        consts = ctx.enter_context(tc.tile_pool(name="consts", bufs=1))
        pers = ctx.enter_context(tc.tile_pool(name="pers", bufs=1))
        wdp = ctx.enter_context(tc.tile_pool(name="wdp", bufs=1))

        # bqk is a tiny-line transfer (48B rows) -> keep it OFF the critical
        # startup path; its DMA is issued after the weight loads below
        bqk_sb = consts.tile([128, 2 * DB], F32, tag="bqk", name="bqk")
        bq_sb = [bqk_sb[:, e:e + 1] for e in range(DB)]
        bk_sb = [bqk_sb[:, DB + e:DB + e + 1] for e in range(DB)]

        ident = consts.tile([128, 128], F16, tag="ident", name="ident")
        make_identity(nc, ident[:])
        eps_sb = consts.tile([128, 1], F32, tag="eps", name="eps")
        nc.gpsimd.memset(eps_sb[:], EPS)
        # bv/bd tiles allocated here; their DMAs are issued after the
        # startup-critical xt chunk 0 loads (same scalar queue). Both are
        # partition-broadcast by gpsimd into full tiles for DVE adds.
        bv_sb = consts.tile([1, D + 2], F16, tag="bv", name="bv")
        bd_sb = consts.tile([1, D], F16, tag="bd", name="bd")
        bv_bc = consts.tile([128, D + 2], F16, tag="bv_bc", name="bv_bc")
        bd_bc = consts.tile([128, D], F16, tag="bd_bc", name="bd_bc")
        # wv|wd packed tiles persist through phase C (wd slices used there)
        wvd_sb = [wdp.tile([128, 2 * D + 2], F16, tag=f"wvd{d}",
                           name=f"wvd{d}") for d in range(DB)]
        wv_sb = [t[:, 0:D + 2] for t in wvd_sb]
        wd_sb = [t[:, D + 2:2 * D + 2] for t in wvd_sb]

        # ---------------- Phase B: K^T, Q^T (both resident), V (resident)
        kt = {}    # (e, chunk) -> [128, SCH] f16 tile
        qt = {}    # (e, chunk) -> [128, SCH] f16 tile
        v_sb = []  # k-block -> [128, 770] bf16 tile
        for k in range(SB):
            v_sb.append(pers.tile([128, D + 2], V_DT, tag=f"v{k}", name=f"v{k}"))

        with tc.tile_pool(name="wqkv", bufs=1) as wpool, \
             tc.tile_pool(name="bx", bufs=2) as bx, \
             tc.tile_pool(name="bkq", bufs=1, space="PSUM") as bkq, \
             tc.tile_pool(name="bpm", bufs=2, space="PSUM") as bpm:
            # parallel DMA queues at startup: wk|wq packed tiles on sync,
            # xt chunk 0 on scalar -> first matmul issues after ~2 tiles
            wkq_sb, xt_first = [], []
            for d in range(DB):
                t = wpool.tile([128, 2 * D], F16, tag=f"wkq{d}", name=f"wkq{d}")
                nc.sync.dma_start(t[:], wkq_d[d])
                wkq_sb.append(t)
                t = bx.tile([128, SCH], F16, tag=f"xt{d}", name=f"xt{d}")
                nc.scalar.dma_start(t[:], xt_d[d, :, 0:SCH])
                xt_first.append(t)
                if d == 0:
                    # small transfer, after the first weight tile so it does
                    # not delay the first matmul; needed by the first evac
                    nc.sync.dma_start(bqk_sb[:], bqk_d[:])
            wk_sb = [t[:, 0:D] for t in wkq_sb]
            wq_sb = [t[:, D:2 * D] for t in wkq_sb]
            for d in range(DB):
                nc.sync.dma_start(wvd_sb[d][:], wvd_d[d])
            nc.scalar.dma_start(bv_sb[:], bv_d[:])
            nc.scalar.dma_start(bd_sb[:], bd_d[:])
            nc.gpsimd.partition_broadcast(bv_bc[:], bv_sb[:])
            nc.gpsimd.partition_broadcast(bd_bc[:], bd_sb[:])

            nsb = SCH // 128  # s-blocks per chunk
            for c in range(NCH):
                if c == 0:
                    xt_c = xt_first
                else:
                    xt_c = []
                    for d in range(DB):
                        t = bx.tile([128, SCH], F16, tag=f"xt{d}", name=f"xt{d}")
                        nc.sync.dma_start(t[:], xt_d[d, :, c * SCH:(c + 1) * SCH])
                        xt_c.append(t)
                # K^T and Q^T e-blocks, d-OUTER with one psum bank per e:
                # matmuls pace with the weight/xt DMA tile arrivals instead
                # of waiting for the full weight set (chunk-0 startup)
                for which in ("k", "q"):
                    w_sb, b_sb = (wk_sb, bk_sb) if which == "k" else (wq_sb, bq_sb)
                    ps = [bkq.tile([128, SCH], F32, tag=f"pe{e}", name=f"pe{e}")
                          for e in range(DB)]
                    for d in range(DB):
                        for e in range(DB):
                            nc.tensor.matmul(ps[e][:],
                                             w_sb[d][:, e * 128:(e + 1) * 128],
                                             xt_c[d][:], start=(d == 0),
                                             stop=(d == DB - 1))
                    for e in range(DB):
                        t = pers.tile([128, SCH], F16, tag=f"{which}t{e}_{c}",
                                      name=f"{which}t{e}_{c}")
                        nc.scalar.activation(t[:], ps[e][:], AF.Relu,
                                             bias=b_sb[e])
                        (kt if which == "k" else qt)[(e, c)] = t
                # V s-blocks (col 768 == 1.0 via bv_aug for softmax row-sums;
                # bv added on DVE from the partition-broadcast row)
                for sb in range(nsb):
                    k_idx = c * nsb + sb
                    for n0, nw in ((0, 512), (512, D + 2 - 512)):
                        pv = bpm.tile([128, 512], F32, tag="pmm", name="pmm")
                        for d in range(DB):
                            nc.tensor.matmul(pv[:, :nw],
                                             xt_c[d][:, sb * 128:(sb + 1) * 128],
                                             wv_sb[d][:, n0:n0 + nw],
                                             start=(d == 0), stop=(d == DB - 1))
                        vt = bx.tile([128, 512], F16, tag="vt", name="vt")
                        nc.vector.tensor_add(vt[:, :nw], pv[:, :nw],
                                             bv_bc[:, n0:n0 + nw])
                        nc.scalar.activation(v_sb[k_idx][:, n0:n0 + nw],
                                             vt[:, :nw], AF.Relu)

        # ------- Phase C (pipelined): scores -> exp -> attn -> LN1 -> proj -> LN2
        nqb = QCH // 128   # q-blocks per chunk
        kt_per_chunk = SCH // 128
        with tc.tile_pool(name="cx", bufs=2) as cx, \
             tc.tile_pool(name="cxr", bufs=1) as cxr, \
             tc.tile_pool(name="cx1", bufs=1) as cx1, \
             tc.tile_pool(name="cx1t", bufs=1) as cx1t, \
             tc.tile_pool(name="cet", bufs=1) as cet, \
             tc.tile_pool(name="cst", bufs=2, space="PSUM") as cst, \
             tc.tile_pool(name="cpa0", bufs=2, space="PSUM") as cpa0, \
             tc.tile_pool(name="cpa1", bufs=1, space="PSUM") as cpa1, \
             tc.tile_pool(name="cpt", bufs=1, space="PSUM") as cpt, \
             tc.tile_pool(name="cpp", bufs=2, space="PSUM") as cpp:

            def load_xres(c):
                x_res = []
                for qs in range(nqb):
                    t = cxr.tile([128, D], F16, tag=f"xr{qs}", name=f"xr{qs}")
                    nc.scalar.dma_start(t[:], xr_d[c * QCH + qs * 128:
                                                   c * QCH + (qs + 1) * 128, :])
                    x_res.append(t)
                return x_res

            def stage_scores(c):
                """S^T = K Q^T per k-block -> E^T = exp(S^T) bf16 (no max
                subtraction: scores < ~72 so exp stays in fp32 range)."""
                et = []
                for k in range(SB):
                    pst = cst.tile([128, QCH], F32, tag="pst", name="pst")
                    for e in range(DB):
                        nc.tensor.matmul(
                            pst[:],
                            kt[(e, k // kt_per_chunk)][
                                :, (k % kt_per_chunk) * 128:
                                   (k % kt_per_chunk + 1) * 128],
                            qt[(e, c)][:], start=(e == 0), stop=(e == DB - 1))
                    et_t = cet.tile([128, QCH], ET_DT, tag=f"et{k}", name=f"et{k}")
                    nc.scalar.activation(et_t[:], pst[:], AF.Exp)
                    et.append(et_t)
                return et

            def attn_mm(et, qs):
                """attn rows + row-sum (col 768) in PSUM for one q-block.
                pa1 (holds the row-sum column) first: the reciprocal and
                the 512:768 residual pass overlap pa0's matmul stream."""
                pa0 = cpa0.tile([128, 512], F32, tag="pa0", name="pa0")
                pa1 = cpa1.tile([128, D + 2 - 512], F32, tag="pa1", name="pa1")
                for k in range(SB):
                    nc.tensor.matmul(pa1[:],
                                     et[k][:, qs * 128:(qs + 1) * 128],
                                     v_sb[k][:, 512:D + 2],
                                     start=(k == 0), stop=(k == SB - 1))
                for k in range(SB):
                    nc.tensor.matmul(pa0[:],
                                     et[k][:, qs * 128:(qs + 1) * 128],
                                     v_sb[k][:, 0:512],
                                     start=(k == 0), stop=(k == SB - 1))
                return pa0, pa1

            def ln1_a1(pa1, x_res, qs):
                """row-sum reciprocal + the 512:768 residual pass (needs
                only pa1, which finishes ~3.4us before pa0)"""
                rcp = cx.tile([128, 1], F32, tag="rcp", name="rcp")
                nc.vector.reciprocal(rcp[:], pa1[:, 256:257])
                r_t = cx.tile([128, D], F32, tag="r_t", name="r_t")
                s1 = cx.tile([128, 1], F32, tag="s1", name="s1")
                nc.vector.scalar_tensor_tensor(
                    r_t[:, 512:D], pa1[:, 0:256], rcp[:], x_res[qs][:, 512:D],
                    op0=ALU.mult, op1=ALU.add, accum_out=s1[:])
                return rcp, r_t, s1

            def ln1_a2(pa0, rcp, r_t, x_res, qs):
                s0 = cx.tile([128, 1], F32, tag="s0", name="s0")
                nc.vector.scalar_tensor_tensor(
                    r_t[:, 0:512], pa0[:], rcp[:], x_res[qs][:, 0:512],
                    op0=ALU.mult, op1=ALU.add, accum_out=s0[:])
                return s0

            def ln1_a(pa0, pa1, x_res, qs):
                rcp, r_t, s1 = ln1_a1(pa1, x_res, qs)
                s0 = ln1_a2(pa0, rcp, r_t, x_res, qs)
                return r_t, s0, s1

            def ln1_b(r_t, s0, s1, tagsfx):
                """LayerNorm -> X1 (fp16, SBUF) (DVE part 2 + ACT sqrt)"""
                negmu = cx.tile([128, 1], F32, tag="negmu", name="negmu")
                nc.vector.tensor_add(negmu[:], s0[:], s1[:])
                nc.vector.tensor_scalar_mul(negmu[:], negmu[:], -1.0 / D)
                # centered accumulate: sum((r-mu)*r) = D*var
                sq_t = cx.tile([128, D], F32, tag="sq_t", name="sq_t", bufs=1)
                dvar = cx.tile([128, 1], F32, tag="dvar", name="dvar")
                nc.vector.scalar_tensor_tensor(
                    sq_t[:], r_t[:], negmu[:], r_t[:],
                    op0=ALU.add, op1=ALU.mult, accum_out=dvar[:])
                sd = cx.tile([128, 1], F32, tag="sd", name="sd")
                nc.scalar.activation(sd[:], dvar[:], AF.Sqrt,
                                     bias=eps_sb[:], scale=1.0 / D)
                rstd = cx.tile([128, 1], F32, tag="rstd", name="rstd")
                nc.vector.reciprocal(rstd[:], sd[:])
                x1 = cx1.tile([128, D], F16, tag=f"x1_{tagsfx}",
                              name=f"x1_{tagsfx}", bufs=1)
                nc.vector.tensor_scalar(x1[:], r_t[:], negmu[:], rstd[:],
                                        op0=ALU.add, op1=ALU.mult)
                return x1

            def attn_qs(c, et, x_res, qs, tagsfx):
                pa0, pa1 = attn_mm(et, qs)
                r_t, s0, s1 = ln1_a(pa0, pa1, x_res, qs)
                return ln1_b(r_t, s0, s1, tagsfx)

            def tp_pe(x1_t, qs):
                # X1^T for one q-block: 6 PE transposes packed into one bank
                pt = cpt.tile([128, DB, 128], F16, tag="pt", name="pt")
                for d in range(DB):
                    nc.tensor.transpose(
                        pt[:, d, :],
                        x1_t[qs][:, d * 128:(d + 1) * 128], ident[:])
                return pt

            def tp_copy(pt, x1t_all, qs):
                # single strided DVE copy into x1t_all
                nc.vector.tensor_copy(
                    x1t_all[:, :, qs * 128:(qs + 1) * 128], pt[:])

            def transpose_qs(x1_t, x1t_all, qs):
                tp_copy(tp_pe(x1_t, qs), x1t_all, qs)

            def dense_mm(x1t_all, qs):
                pp = []
                for n0, nw in ((0, 384), (384, 384)):
                    p = cpp.tile([128, 384], F32, tag="pp", name="pp")
                    for d in range(DB):
                        nc.tensor.matmul(p[:],
                                         x1t_all[:, d, qs * 128:(qs + 1) * 128],
                                         wd_sb[d][:, n0:n0 + nw],
                                         start=(d == 0), stop=(d == DB - 1))
                    pp.append(p)
                return pp

            def dense_ln2(c, pp, x1_t, qs):
                # +x1+bd residual -> LN2 -> out rows
                x1bd = cx.tile([128, D], F16, tag="x1bd", name="x1bd")
                nc.vector.tensor_add(x1bd[:], x1_t[qs][:], bd_bc[:])
                y_t = cx.tile([128, D], F32, tag="y_t", name="y_t")
                t0 = cx.tile([128, 1], F32, tag="t0", name="t0")
                t1 = cx.tile([128, 1], F32, tag="t1", name="t1")
                nc.vector.scalar_tensor_tensor(
                    y_t[:, 0:384], pp[0][:], 0.0, x1bd[:, 0:384],
                    op0=ALU.add, op1=ALU.add, accum_out=t0[:])
                nc.vector.scalar_tensor_tensor(
                    y_t[:, 384:D], pp[1][:], 0.0, x1bd[:, 384:D],
                    op0=ALU.add, op1=ALU.add, accum_out=t1[:])
                negmu2 = cx.tile([128, 1], F32, tag="negmu2", name="negmu2")
                nc.vector.tensor_add(negmu2[:], t0[:], t1[:])
                nc.vector.tensor_scalar_mul(negmu2[:], negmu2[:], -1.0 / D)
                sq2 = cx.tile([128, D], F32, tag="sq2", name="sq2", bufs=1)
                dvar2 = cx.tile([128, 1], F32, tag="dvar2", name="dvar2")
                nc.vector.scalar_tensor_tensor(
                    sq2[:], y_t[:], negmu2[:], y_t[:],
                    op0=ALU.add, op1=ALU.mult, accum_out=dvar2[:])
                sd2 = cx.tile([128, 1], F32, tag="sd2", name="sd2")
                nc.scalar.activation(sd2[:], dvar2[:], AF.Sqrt,
                                     bias=eps_sb[:], scale=1.0 / D)
                rstd2 = cx.tile([128, 1], F32, tag="rstd2", name="rstd2")
                nc.vector.reciprocal(rstd2[:], sd2[:])
                out_t = cx.tile([128, D], F32, tag="out_t", name="out_t")
                nc.vector.tensor_scalar(out_t[:], y_t[:], negmu2[:],
                                        rstd2[:], op0=ALU.add, op1=ALU.mult)
                r0 = c * QCH + qs * 128
                nc.sync.dma_start(out_d[r0:r0 + 128, :], out_t[:])

            def dense_qs(c, x1_t, x1t_all, qs):
                pp = dense_mm(x1t_all, qs)
                dense_ln2(c, pp, x1_t, qs)

            # software pipeline: interleave attn/transpose/dense per q-block
            # so PE matmuls cover every LN chain and every transpose->copy
            # latency; scores(c+1) covers the chunk's tail. Output DMA
            # stays spread across the whole phase (deferring it drains the
            # DMA queue at the end -- measured worse).
            xres_cur = load_xres(0)
            et_cur = stage_scores(0)
            for c in range(NCH):
                if c + 1 < NCH:
                    xres_nxt = load_xres(c + 1)
                x1_t = [None] * nqb
                x1t_all = cx1t.tile([128, DB, QCH], F16, tag="x1t",
                                    name="x1t", bufs=1)
                x1_t[0] = attn_qs(c, et_cur, xres_cur, 0, "0")
                x1_t[1] = attn_qs(c, et_cur, xres_cur, 1, "1")
                transpose_qs(x1_t, x1t_all, 0)
                x1_t[2] = attn_qs(c, et_cur, xres_cur, 2, "2")
                dense_qs(c, x1_t, x1t_all, 0)
                transpose_qs(x1_t, x1t_all, 1)
                x1_t[3] = attn_qs(c, et_cur, xres_cur, 3, "3")
                dense_qs(c, x1_t, x1t_all, 1)
                transpose_qs(x1_t, x1t_all, 2)
                if c + 1 < NCH:
                    et_cur = stage_scores(c + 1)
                    xres_cur = xres_nxt
                dense_qs(c, x1_t, x1t_all, 2)
                transpose_qs(x1_t, x1t_all, 3)
                dense_qs(c, x1_t, x1t_all, 3)

    _split_matmul_waits(nc)
    nc.compile()
    return nc


_NC_CACHE = None


def _get_nc():
    global _NC_CACHE
    if _NC_CACHE is None:
        _NC_CACHE = _build()
    return _NC_CACHE


def _prep_in_maps(X, Wq, bq, Wk, bk, Wv, bv, Wd, bd):
    X = np.ascontiguousarray(X, np.float32)
    f16 = np.float16
    wq = np.asarray(Wq, np.float32).astype(f16).reshape(DB, 128, D)
    wk = np.asarray(Wk, np.float32).astype(f16).reshape(DB, 128, D)
    wv_aug = np.zeros((D, D + 2), f16)
    wv_aug[:, :D] = np.asarray(Wv, np.float32).astype(f16)
    wv_aug = wv_aug.reshape(DB, 128, D + 2)
    wd = np.asarray(Wd, np.float32).astype(f16).reshape(DB, 128, D)
    wkq = np.ascontiguousarray(np.concatenate([wk, wq], axis=2))
    wvd = np.ascontiguousarray(np.concatenate([wv_aug, wd], axis=2))
    bv_aug = np.zeros((1, D + 2), f16)
    bv_aug[0, :D] = np.asarray(bv, np.float32).astype(f16)
    bv_aug[0, D] = 1.0
    bd_r = np.asarray(bd, np.float32).astype(f16).reshape(1, D)
    shared = {
        "wkq": wkq, "wvd": wvd,
        "bqk": np.ascontiguousarray(np.concatenate(
            [np.asarray(bq, np.float32).reshape(DB, 128, 1),
             np.asarray(bk, np.float32).reshape(DB, 128, 1)], axis=0)
            .transpose(1, 0, 2).reshape(128, 2 * DB)),
        "bv": bv_aug, "bd": bd_r,
    }
    return [dict(shared,
                 xr=X[c].astype(f16),
                 xt=np.ascontiguousarray(X[c].T).astype(f16).reshape(DB, 128, S))
            for c in range(N_CORES)]


def _run(inputs, trace=False, trace_kwargs=None):
    in_maps = _prep_in_maps(
        inputs["X"], inputs["Wq"], inputs["bq"], inputs["Wk"], inputs["bk"],
        inputs["Wv"], inputs["bv"], inputs["Wd"], inputs["bd"])
    nc = _get_nc()
    res = run_bass_kernel_spmd(nc, in_maps, list(range(N_CORES)),
                               trace=trace, **(trace_kwargs or {}))
    out = np.stack([res.results[c]["out"] for c in range(N_CORES)])
    return out, res


def kernel(X, Wq, bq, Wk, bk, Wv, bv, Wd, bd, g1, b1, g2, b2):
    out, _ = _run(dict(X=X, Wq=Wq, bq=bq, Wk=Wk, bk=bk, Wv=Wv, bv=bv,
                       Wd=Wd, bd=bd))
    g1 = np.asarray(g1); b1 = np.asarray(b1)
    g2 = np.asarray(g2); b2 = np.asarray(b2)
    # The kernel folds the (identity) LN affines away; handle the general
    # case anyway. A non-identity g1/b1 feeds the dense layer and cannot be
    # patched after the fact -> recompute on host (never hit for this
    # problem's deterministic inputs: g=1, b=0).
    if not (np.allclose(g1, 1.0) and np.allclose(b1, 0.0)):
        return _host_reference(X, Wq, bq, Wk, bk, Wv, bv, Wd, bd, g1, b1, g2, b2)
    if not (np.allclose(g2, 1.0) and np.allclose(b2, 0.0)):
        out = out * np.asarray(g2) + np.asarray(b2)
    return out.astype(np.float32)


def _host_reference(X, Wq, bq, Wk, bk, Wv, bv, Wd, bd, g1, b1, g2, b2):
    X = np.asarray(X, np.float64)
    out = np.empty_like(X)
    for c in range(X.shape[0]):
        x = X[c]
        Q = np.maximum(x @ Wq + bq, 0)
        K = np.maximum(x @ Wk + bk, 0)
        V = np.maximum(x @ Wv + bv, 0)
        Sc = Q @ K.T
        Sc -= Sc.max(-1, keepdims=True)
        E = np.exp(Sc)
        A = (E @ V) / E.sum(-1, keepdims=True)
        X1 = x + A
        X1 = (X1 - X1.mean(-1, keepdims=True)) / np.sqrt(
            X1.var(-1, keepdims=True) + EPS) * g1 + b1
        X2 = X1 + X1 @ Wd + bd
        X2 = (X2 - X2.mean(-1, keepdims=True)) / np.sqrt(
            X2.var(-1, keepdims=True) + EPS) * g2 + b2
        out[c] = X2
    return out.astype(np.float32)
